# revision 3
# baseline (speedup 1.0000x reference)
"""Trainium2 kernel for IterativeEdgeModel.assign_clusters (8 NeuronCores).

Sharding strategy: each of the 8 cores owns a contiguous range of 12,500
destination nodes. During sharding, edges are routed to their owner core and
laid out CSR-style: a dense [nodes, L] grid (stable order, so free-dim
position = ascending global edge index), padded with pred=-1. The device
kernel then needs no data-dependent addressing at all:

  per 128-node tile (3 fused DVE instructions over [128, L]):
    best[n] = reduce_max(pred[n, :])                    # segment_max
    eqr     = (pred == best) * revj,  revj[j] = L - j   # mark max positions
    r[n]    = reduce_max(eqr)                           # = L - (first argmax j)

Host post-processing is all O(N): j* = L - r, win_src = src at (n, j*),
win_pred = max(best, 0), scatter update of `matched` — identical to the
reference's final jnp.where.
"""

import os
import numpy as np

N_NODES = 200_000
HALF = 100_000
N_CORES = 8
NODES_PER_CORE = HALF // N_CORES          # 12_500
NBLK = (NODES_PER_CORE + 127) // 128      # 98 blocks of 128 nodes
NPAD = NBLK * 128                         # 12_544 padded nodes per core
L = 192                                   # padded per-node edge slots
GRP = 7                                   # blocks per input DMA (98 = 14*7)
THRESH = 0.5

_CACHE = {}
LAST_RESULT = None  # BassKernelResults of the most recent run (for profiling)


def _build_nc():
    import concourse.bass as bass
    import concourse.mybir as mybir
    from concourse import bacc, tile

    f32 = mybir.dt.float32
    nc = bacc.Bacc(None, target_bir_lowering=False, debug=False)

    pred_d = nc.dram_tensor("pred", [128, NBLK * L], f32, kind="ExternalInput")
    best_d = nc.dram_tensor("best", [128, NBLK], f32, kind="ExternalOutput")
    r_d = nc.dram_tensor("r", [128, NBLK], f32, kind="ExternalOutput")

    revj_np = np.tile((L - np.arange(L, dtype=np.float32))[None, :], (128, 1))
    revj_d = nc.inline_tensor(revj_np, name="revj")

    with tile.TileContext(nc) as tc:
        with (
            tc.tile_pool(name="io", bufs=3) as iop,
            tc.tile_pool(name="small", bufs=1) as sp,
            tc.tile_pool(name="tmp", bufs=2) as tp,
        ):
            revj = sp.tile([128, L], f32)
            nc.sync.dma_start(revj[:], revj_d[:])
            best_acc = sp.tile([128, NBLK], f32)
            r_acc = sp.tile([128, NBLK], f32)

            eq = mybir.AluOpType.is_equal
            mult = mybir.AluOpType.mult
            mx = mybir.AluOpType.max
            X = mybir.AxisListType.X

            for g in range(NBLK // GRP):
                pt = iop.tile([128, GRP * L], f32, tag="pred")
                nc.sync.dma_start(pt[:], pred_d[:, g * GRP * L:(g + 1) * GRP * L])
                scr = tp.tile([128, GRP * L], f32, tag="scr")
                for b in range(GRP):
                    col = g * GRP + b
                    sl = slice(b * L, (b + 1) * L)
                    nc.vector.tensor_reduce(
                        best_acc[:, col:col + 1], pt[:, sl], axis=X, op=mx)
                    nc.vector.scalar_tensor_tensor(
                        scr[:, sl], pt[:, sl], best_acc[:, col:col + 1],
                        revj[:], op0=eq, op1=mult)
                    nc.vector.tensor_reduce(
                        r_acc[:, col:col + 1], scr[:, sl], axis=X, op=mx)
            nc.sync.dma_start(best_d[:], best_acc[:])
            nc.sync.dma_start(r_d[:], r_acc[:])
    nc.finalize()
    return nc


def kernel(edge_pred, edge_index, matched):
    from concourse.bass_utils import run_bass_kernel_spmd
    global LAST_RESULT

    edge_pred = np.ascontiguousarray(np.asarray(edge_pred, dtype=np.float32))
    edge_index = np.asarray(edge_index)
    matched = np.asarray(matched)
    E = edge_pred.shape[0]
    src = edge_index[0].astype(np.int64, copy=False)
    dst = edge_index[1].astype(np.int64, copy=False)

    # ---- shard: route edges to owner core by dst range, CSR-pad to [NPAD, L]
    dst_rel = (dst - HALF).astype(np.int32)
    order = np.argsort(dst_rel, kind="stable")        # radix sort, stable
    dst_sorted = dst_rel[order]
    counts = np.bincount(dst_rel, minlength=HALF)
    starts = np.zeros(HALF, dtype=np.int64)
    np.cumsum(counts[:-1], out=starts[1:])
    pos = np.arange(E, dtype=np.int64) - starts[dst_sorted]  # rank within node
    keep = pos < L
    overflow_nodes = np.unique(dst_sorted[~keep]) if not keep.all() else None

    # global padded grid [N_CORES*NPAD, L]; node n -> row (n//12500 core pad)
    core_of = dst_sorted // NODES_PER_CORE
    row = core_of * NPAD + (dst_sorted - core_of * NODES_PER_CORE)
    slot = row * L + pos
    predpad = np.full(N_CORES * NPAD * L, -1.0, dtype=np.float32)
    predpad[slot[keep]] = edge_pred[order[keep]]
    predpad = predpad.reshape(N_CORES, NBLK, 128, L)

    in_maps = []
    for c in range(N_CORES):
        # -> partition-major [128, NBLK*L]
        pc = np.ascontiguousarray(
            predpad[c].transpose(1, 0, 2).reshape(128, NBLK * L))
        in_maps.append({"pred": pc})

    # ---- run on 8 NeuronCores
    if "nc" not in _CACHE:
        _CACHE["nc"] = _build_nc()
    trace = bool(os.environ.get("BASS_TRACE"))
    try:
        res = run_bass_kernel_spmd(
            _CACHE["nc"], in_maps, core_ids=list(range(N_CORES)), trace=trace)
    except ModuleNotFoundError:
        os.environ["BASS_NEVER_TRACE"] = "1"
        res = run_bass_kernel_spmd(
            _CACHE["nc"], in_maps, core_ids=list(range(N_CORES)), trace=False)
    LAST_RESULT = res

    best = np.empty(HALF, dtype=np.float32)
    rr = np.empty(HALF, dtype=np.float32)
    for c in range(N_CORES):
        b = res.results[c]["best"]          # [128, NBLK]
        r = res.results[c]["r"]
        sl = slice(c * NODES_PER_CORE, (c + 1) * NODES_PER_CORE)
        best[sl] = b.T.reshape(-1)[:NODES_PER_CORE]
        rr[sl] = r.T.reshape(-1)[:NODES_PER_CORE]

    # ---- host unshard/postprocess (all O(N))
    has_edge = best >= 0.0
    jstar = (L - rr).astype(np.int64)
    jstar_c = np.clip(jstar, 0, None)
    # win edge: j-th (stable-order) edge of node n  -> global edge id
    nsafe = np.arange(HALF, dtype=np.int64)
    eidx = starts + np.minimum(jstar_c, np.maximum(counts - 1, 0))
    win_src = src[order[eidx]]              # valid only where has_edge

    if overflow_nodes is not None and overflow_nodes.size:
        for n in overflow_nodes:            # exact host fallback (rare)
            ids = order[starts[n]:starts[n] + counts[n]]
            p = edge_pred[ids]
            bm = p.max()
            j = int(np.argmax(p == bm))
            best[n] = bm
            win_src[n] = src[ids[j]]
        has_edge = best >= 0.0

    win_pred_half = np.where(has_edge, best, np.float32(0.0)).astype(np.float32)
    win_pred = np.zeros(N_NODES, dtype=np.float32)
    win_pred[HALF:] = win_pred_half

    update_half = (matched[HALF:] == -1) & has_edge & (win_pred_half > THRESH)
    new_matched = matched.copy()
    upd_src = win_src[update_half]
    new_matched[HALF:][update_half] = matched[upd_src]
    # nodes < HALF have no incoming edges in this graph; if they did, the
    # generic path below would handle them (kept simple: dst >= HALF per spec)
    found = bool(update_half.any())
    return (new_matched.astype(matched.dtype, copy=False),
            win_pred,
            np.bool_(found))


# revision 6
# speedup vs baseline: 2.3866x; 2.3866x over previous
"""Trainium2 kernel for IterativeEdgeModel.assign_clusters (8 NeuronCores).

Sharding: each core owns a contiguous range of 12,500 destination nodes.
While sharding, edges are routed to their owner core and laid out CSR-style
in a dense per-node grid (stable order, so the free-dim position j equals
the rank of the edge's global index within its node). The device then needs
no data-dependent addressing. Nodes are degree-sorted into blocks of 128 so
each group of 7 blocks gets a tight compile-time row length L_g (cuts ~30%
padding vs a global max degree).

Device kernel (exact for arbitrary non-negative f32 pred), per group of 7
blocks, all batched DVE instructions:
  1. chunk-reduce   Pm[n,c] = max(pred[n, 32c:32c+32])     (the only full pass)
  2. best[n]        = max_c Pm[n,c]                        (exact segment max)
  3. eq   = (Pm == best)                                   (broadcast compare)
  4. eqr  = eq * revc,  revc[c] = CMAX - c
  5. r2[n] = max_c eqr  ->  c* = CMAX - r2 = first chunk attaining best

Host post-processing: scans only the winning 32-wide chunk per node for the
first j with pred == best (exact argmin tie-break), then the O(N) scatter
update of `matched` identical to the reference's final where().
"""

import os
import numpy as np

N_NODES = 200_000
HALF = 100_000
N_CORES = 8
NODES_PER_CORE = HALF // N_CORES          # 12_500
NBLK = (NODES_PER_CORE + 127) // 128      # 98 blocks of 128 nodes
NPAD = NBLK * 128                         # 12_544 padded nodes per core
GRP = 7                                   # blocks per DMA/reduce group
NGRP = NBLK // GRP                        # 14
CHUNK = 32
LMAX = 256                                # cap; deg > LMAX -> host fallback
CMAX = LMAX // CHUNK                      # 8
PAD = np.float32(-1.0)
THRESH = 0.5

_CACHE = {}
LAST_RESULT = None  # BassKernelResults of the most recent run (for profiling)


def _build_nc_chunked(l_gs):
    """l_gs: per-group row lengths (multiples of CHUNK)."""
    import concourse.mybir as mybir
    from concourse import bacc, tile

    f32 = mybir.dt.float32
    totcol = sum(GRP * lg for lg in l_gs)
    nc = bacc.Bacc(None, target_bir_lowering=False, debug=False)
    pred_d = nc.dram_tensor("pred", [128, totcol], f32, kind="ExternalInput")
    best_d = nc.dram_tensor("best", [128, NBLK], f32, kind="ExternalOutput")
    r2_d = nc.dram_tensor("r2", [128, NBLK], f32, kind="ExternalOutput")
    revc_np = np.tile((CMAX - np.arange(CMAX, dtype=np.float32))[None, :],
                      (128, 1))
    revc_d = nc.inline_tensor(revc_np, name="revc")

    X = mybir.AxisListType.X
    mx = mybir.AluOpType.max
    eq = mybir.AluOpType.is_equal
    mult = mybir.AluOpType.mult

    with tile.TileContext(nc) as tc:
        with (
            tc.tile_pool(name="io", bufs=4) as iop,
            tc.tile_pool(name="pm", bufs=3) as pmp,
            tc.tile_pool(name="small", bufs=1) as sp,
        ):
            revc = sp.tile([128, CMAX], f32)
            nc.sync.dma_start(revc[:], revc_d[:])
            best_acc = sp.tile([128, NBLK], f32)
            r2_acc = sp.tile([128, NBLK], f32)
            off = 0
            for g, lg in enumerate(l_gs):
                w = GRP * lg
                cg = lg // CHUNK
                gs = slice(g * GRP, (g + 1) * GRP)
                pt = iop.tile([128, w], f32, tag="pred")
                nc.sync.dma_start(pt[:], pred_d[:, off:off + w])
                pm = pmp.tile([128, GRP * cg], f32, tag="pm")
                pm3 = pm[:].rearrange("p (b c) -> p b c", b=GRP)
                nc.vector.tensor_reduce(
                    pm3, pt[:].rearrange("p (b c s) -> p b c s", b=GRP, s=CHUNK),
                    axis=X, op=mx)
                nc.vector.tensor_reduce(best_acc[:, gs], pm3, axis=X, op=mx)
                eqt = pmp.tile([128, GRP * cg], f32, tag="eq")
                eq3 = eqt[:].rearrange("p (b c) -> p b c", b=GRP)
                bb = best_acc[:, gs].unsqueeze(2).broadcast_to([128, GRP, cg])
                nc.vector.tensor_tensor(eq3, pm3, bb, op=eq)
                rc = revc[:, :cg].unsqueeze(1).broadcast_to([128, GRP, cg])
                nc.vector.tensor_tensor(eq3, eq3, rc, op=mult)
                nc.vector.tensor_reduce(r2_acc[:, gs], eq3, axis=X, op=mx)
                off += w
            nc.sync.dma_start(best_d[:], best_acc[:])
            nc.sync.dma_start(r2_d[:], r2_acc[:])
    nc.finalize()
    return nc


def _run(nc, in_maps):
    from concourse.bass_utils import run_bass_kernel_spmd
    global LAST_RESULT
    trace = bool(os.environ.get("BASS_TRACE"))
    try:
        res = run_bass_kernel_spmd(
            nc, in_maps, core_ids=list(range(N_CORES)), trace=trace)
    except ModuleNotFoundError:
        os.environ["BASS_NEVER_TRACE"] = "1"
        res = run_bass_kernel_spmd(
            nc, in_maps, core_ids=list(range(N_CORES)), trace=False)
    LAST_RESULT = res
    return res


def kernel(edge_pred, edge_index, matched):
    edge_pred = np.ascontiguousarray(np.asarray(edge_pred, dtype=np.float32))
    edge_index = np.asarray(edge_index)
    matched = np.asarray(matched)
    E = edge_pred.shape[0]
    src = edge_index[0].astype(np.int64, copy=False)
    dst = edge_index[1].astype(np.int64, copy=False)

    # ---- CSR by dst (stable => within-node order == ascending edge index)
    dst_rel = (dst - HALF).astype(np.int32)
    order = np.argsort(dst_rel, kind="stable")
    dst_sorted = dst_rel[order]
    counts = np.bincount(dst_rel, minlength=HALF).astype(np.int64)
    starts = np.zeros(HALF, dtype=np.int64)
    np.cumsum(counts[:-1], out=starts[1:])
    pos = np.arange(E, dtype=np.int64) - starts[dst_sorted]

    if edge_pred.min() >= 0.0:
        best, win_src, has_edge = _segment_device(
            edge_pred, src, order, dst_sorted, counts, starts, pos)
    else:  # never hit for this model (pred = uniform[0,1)); exact host path
        best, win_src, has_edge = _segment_host(edge_pred, src, dst_rel, E)

    # ---- host unshard/postprocess (all O(N))
    win_pred_half = np.where(has_edge, best, np.float32(0.0)).astype(np.float32)
    win_pred = np.zeros(N_NODES, dtype=np.float32)
    win_pred[HALF:] = win_pred_half
    update_half = (matched[HALF:] == -1) & has_edge & (win_pred_half > THRESH)
    new_matched = matched.copy()
    new_matched[HALF:][update_half] = matched[win_src[update_half]]
    found = bool(update_half.any())
    return (new_matched.astype(matched.dtype, copy=False),
            win_pred,
            np.bool_(found))


def _segment_device(edge_pred, src, order, dst_sorted, counts, starts, pos):
    # degree-sort nodes within each core; per-group row lengths (shared
    # across cores so the SPMD program has one set of shapes)
    counts_c = counts.reshape(N_CORES, NODES_PER_CORE)
    perm = np.argsort(-counts_c, axis=1, kind="stable")     # [8, 12500]
    perm_pos = np.empty_like(perm)
    ar = np.arange(NODES_PER_CORE, dtype=np.int64)[None, :]
    np.put_along_axis(perm_pos, perm, np.broadcast_to(ar, perm.shape), axis=1)
    sorted_cnt = np.take_along_axis(counts_c, perm, axis=1)
    sorted_cnt_pad = np.zeros((N_CORES, NPAD), dtype=np.int64)
    sorted_cnt_pad[:, :NODES_PER_CORE] = sorted_cnt
    blk_max = sorted_cnt_pad.reshape(N_CORES, NBLK, 128).max(axis=2)
    l_bs = np.minimum(blk_max.max(axis=0), LMAX)            # [NBLK] over cores
    l_gs = [int(l_bs[g * GRP:(g + 1) * GRP].max()) for g in range(NGRP)]
    l_gs = [max(CHUNK, -(-lg // CHUNK) * CHUNK) for lg in l_gs]
    cstart = np.zeros(NGRP, dtype=np.int64)
    np.cumsum([GRP * lg for lg in l_gs[:-1]], out=cstart[1:])
    totcol = int(cstart[-1] + GRP * l_gs[-1])
    l_g_of_blk = np.repeat(np.asarray(l_gs, dtype=np.int64), GRP)
    colbase_blk = cstart[np.arange(NBLK) // GRP] \
        + (np.arange(NBLK) % GRP) * l_g_of_blk              # [NBLK]

    # per-edge destination slot in the [8, 128, totcol] grid
    core = dst_sorted // NODES_PER_CORE
    s = perm_pos[core, dst_sorted - core * NODES_PER_CORE]  # degree-sorted idx
    b = s // 128
    p = s % 128
    keep = pos < l_g_of_blk[b]
    kid = np.nonzero(keep)[0]
    flat = (core[kid] * 128 + p[kid]) * totcol + colbase_blk[b[kid]] + pos[kid]
    grid = np.full(N_CORES * 128 * totcol, PAD, dtype=np.float32)
    grid[flat] = edge_pred[order[kid]]
    grid = grid.reshape(N_CORES, 128, totcol)

    key = ("chunked", tuple(l_gs))
    if key not in _CACHE:
        _CACHE[key] = _build_nc_chunked(l_gs)
    res = _run(_CACHE[key], [{"pred": grid[c]} for c in range(N_CORES)])

    best = np.empty(HALF, dtype=np.float32)
    cstar = np.empty(HALF, dtype=np.int64)
    for c in range(N_CORES):
        bs = res.results[c]["best"].T.reshape(-1)[:NODES_PER_CORE]
        r2 = res.results[c]["r2"].T.reshape(-1)[:NODES_PER_CORE]
        sl = slice(c * NODES_PER_CORE, (c + 1) * NODES_PER_CORE)
        bn = np.empty_like(bs)                              # bs[s] is perm[c,s]
        bn[perm[c]] = bs
        best[sl] = bn
        cn = np.empty_like(r2)
        cn[perm[c]] = r2
        cstar[sl] = CMAX - cn.astype(np.int64)

    has_edge = (counts > 0) & (best > PAD)
    # host tie-break: first pred == best inside the winning 32-chunk
    pred_csr = edge_pred[order]
    lo = starts + cstar * CHUNK
    win = np.nonzero(has_edge)[0]
    idx = lo[win, None] + np.arange(CHUNK, dtype=np.int64)[None, :]
    valid = idx < (starts[win] + counts[win])[:, None]
    vals = pred_csr[np.minimum(idx, pred_csr.size - 1)]
    match = valid & (vals == best[win, None])
    j_loc = np.argmax(match, axis=1)
    win_src = np.zeros(HALF, dtype=np.int64)
    win_src[win] = src[order[lo[win] + j_loc]]

    over = np.nonzero(counts > LMAX)[0]
    if over.size:
        for n in over:                                      # exact, rare
            ids = order[starts[n]:starts[n] + counts[n]]
            pv = edge_pred[ids]
            bm = pv.max()
            best[n] = bm
            win_src[n] = src[ids[int(np.argmax(pv == bm))]]
        has_edge[over] = True
    return best, win_src, has_edge


def _segment_host(edge_pred, src, dst_rel, E):
    """Pure-numpy exact path (only for inputs outside the model's domain)."""
    best = np.full(HALF, -np.inf, dtype=np.float32)
    np.maximum.at(best, dst_rel, edge_pred)
    cand = np.where(edge_pred == best[dst_rel], np.arange(E, dtype=np.int64), E)
    win_edge = np.full(HALF, E, dtype=np.int64)
    np.minimum.at(win_edge, dst_rel, cand)
    has_edge = win_edge < E
    win_src = src[np.where(has_edge, win_edge, 0)]
    return np.where(has_edge, best, 0).astype(np.float32), win_src, has_edge
